# revision 14
# baseline (speedup 1.0000x reference)
"""AdaptiveGraphLearning forward on 8 Trainium2 NeuronCores.

Data-parallel over batch B=64: each core processes 8 batches; the (N,N)
adjacency parameter and tiny edge-MLP weights are replicated (the forward
pass needs no collectives).

v3 dataflow (per core, 8 batches as 4 pairs). v1 was TensorE-bound (87%
busy: X = Wi.T@node_i + Wj.T@node_j as 512-wide broadcast-moving
matmuls). v3 computes tiny projections pi/pj once per pair (256 streamed
columns instead of 8192) and materializes X = relu(pi_i + pj_j) on the
vector engines, leaving PE only the irreducible W2/W3 streams:

  - DMA all pairs as 2MB h-quarter tiles on the SP + ACT HWDGE rings
    (sustains ~430 GB/s; quarters keep the t-fold within ~2us of data
    landing). Consts ride ahead as 2 packed transfers.
  - Sum over t: f32->bf16 fold then bf16 folds (2x DVE mode; all
    operands SBUF, unit stride). Quarters split DVE/Pool per ENG_QUARTER
    (Pool's software ALU is ~2.1 ns/el, so it gets a bounded share; its
    chain is pure tensor_tensor folds to t=1 since Pool lacks axis-X
    reduce).
  - R (128=(b_lo,n), 128=h) bf16 -> node_T via DMA XBAR transpose.
  - pi/pj: 2 matmuls of 128 cols; ACT evacuates (b1 folded into pi).
  - Per chunk (8 i x 64 j, both batches = 1024 cols): X-add on DVE/Pool
    (broadcast APs; stride-0 innermost keeps this 1x), relu on ACT
    (SBUF->SBUF), PE W2 (2x512 cols) -> h2 relu evac (+b2) on ACT/Pool
    -> PE W3 one-hot accumulate into per-batch (8,512) PSUM.
  - Epilogue per batch: F(8,512) -> (64,64) via SBUF DMA reshape, F^T on
    PE, out = (relu(G + F + F^T) + I) / row-sum with G = 0.25*(ap+ap^T)
    precomputed host-side (0.25 sym factor folded into W3/b3).

Harness notes: walrus in this container accepts a single semaphore wait
per instruction, so a BIR-level pass splits Tile's multi-wait
instructions onto EventSemaphore carriers; the Tile kernel-tail drain
gets the same treatment at build time.
"""
import sys

sys.path.insert(0, '/opt/trn_rl_repo')

import numpy as np

B, N, H, T = 64, 64, 128, 128
NCORES = 8
B_LOC = B // NCORES      # 8 batches per core
PAIRS = B_LOC // 2       # 4 batch pairs per core
NCH = N // 8             # 8 i-chunks per batch (8 i x 64 j = 512 wide)
HQ = H // 4              # quarter h-range per DMA tile

# packed const layouts (columns)
CB_WI, CB_WJ, CB_W2, CB_W3 = 0, 128, 256, 320   # bf16 block, width 384
CF_B1, CF_B2, CF_B3, CF_G, CF_I64, CF_Z = 0, 1, 2, 3, 67, 131  # f32, w 132

# --- engine assignment knobs (tuned from traces) ---
# fold engine per quarter index: 'v' = DVE chain, 'g1' = Pool does the
# big f32 fold1, DVE the bf16 tail (Pool chains are slow; one f32 fold1
# per pair balances it)
ENG_QUARTER = ('v', 'v', 'v', 'g1')
# X-add engine per chunk
XADD_ENG = ('v', 'v', 'g', 'g', 'g', 'g', 'g', 'g')
# h2 evacuation engine per chunk ('a' = ACT; 'v' = DVE tensor_scalar)
H2_ENG = ('a', 'a', 'a', 'a', 'a', 'a', 'a', 'a')
# ff evacuation: 'v' = DVE tensor_scalar (PSUM-capable), 'a' = ACT
FF_ENG = 'v'

_CACHE = {}


def _install_wait_splitter():
    """walrus's per-instruction sync structs hold a single semaphore wait;
    Tile can emit several on one instruction. Split extras onto preceding
    single-wait EventSemaphore instructions at the BIR-JSON level."""
    if _CACHE.get('wait_splitter'):
        return
    import json

    import concourse.bass2jax as bass2jax

    orig = bass2jax.compile_bir_kernel

    def split_waits_in_bir(bir_bytes):
        d = json.loads(bir_bytes)
        n_new = [0]
        for fn in d.get("functions", []):
            for bb in fn.get("blocks", []):
                out = []
                for ins in bb.get("instructions", []):
                    si = ins.get("sync_info") or {}
                    waits = si.get("on_wait") or []
                    if len(waits) > 1:
                        for w in waits[:-1]:
                            n_new[0] += 1
                            out.append({
                                "engine": ins["engine"],
                                "ins": [],
                                "name": f"IWS-{n_new[0]}",
                                "opcode": "EventSemaphore",
                                "outs": [],
                                "sync_info": {"on_update": [], "on_wait": [w]},
                            })
                        si["on_wait"] = [waits[-1]]
                    out.append(ins)
                bb["instructions"] = out
        return json.dumps(d).encode()

    def wrapper(ant_bir_str, *a, **kw):
        return orig(split_waits_in_bir(ant_bir_str), *a, **kw)

    bass2jax.compile_bir_kernel = wrapper
    _CACHE['wait_splitter'] = True


def _split_drain_tile_context(tile_mod, mybir, nc):
    """TileContext whose kernel-tail drain splits its semaphore waits across
    sequential Drain instructions (walrus CTRL insts accept one wait)."""
    from concourse.tile import ScopedClock

    class SplitDrainTileContext(tile_mod.TileContext):
        def _drain_and_barrier(self, tick_clock, wait_clock):
            drain_inst = self.nc.sync.drain()
            wait_clock.add_sem_waits(
                drain_inst.ins, ScopedClock({None: tick_clock.global_clock})
            )
            waits = list(drain_inst.ins.sync_info.on_wait)
            if len(waits) > 1:
                drain_inst.ins.sync_info = mybir.SyncInfo(
                    on_wait=waits[:1],
                    on_update=list(drain_inst.ins.sync_info.on_update),
                )
                for i in range(1, len(waits)):
                    extra = self.nc.sync.drain()
                    extra.ins.sync_info = mybir.SyncInfo(
                        on_wait=waits[i : i + 1], on_update=[]
                    )
            self.nc.all_engine_barrier()
            assert self.sems is not None
            popped = self.nc._tile_sem_poison_stack.pop()
            assert popped is self._sem_poison
            self.nc.clear_and_free_semaphores(list(self.sems.allocated().values()))
            self.nc.all_engine_barrier()

    return SplitDrainTileContext(nc)


def build_nc():
    import concourse.bass as bass
    import concourse.tile as tile
    from concourse import mybir
    from contextlib import ExitStack

    f32 = mybir.dt.float32
    bf16 = mybir.dt.bfloat16
    AF = mybir.ActivationFunctionType
    ALU = mybir.AluOpType
    AX = mybir.AxisListType

    nc = bass.Bass()
    tf = nc.declare_dram_parameter("tf", [B_LOC, N, H, T], f32, isOutput=False)
    CB = nc.declare_dram_parameter("CB", [128, 384], bf16, isOutput=False)
    CF = nc.declare_dram_parameter("CF", [128, 132], f32, isOutput=False)
    out_ext = nc.declare_dram_parameter("out", [B_LOC, N, N], f32, isOutput=True)

    NOBIAS = _CACHE.get('cfg_nobias', False)

    with _split_drain_tile_context(tile, mybir, nc) as tc, ExitStack() as ctx, \
            nc.allow_low_precision("bf16 t-fold accumulation within 2e-2 tol"):
        consts = ctx.enter_context(tc.tile_pool(name="consts", bufs=1))
        tf_pool = ctx.enter_context(tc.tile_pool(name="tf", bufs=7))
        fold_pool = ctx.enter_context(tc.tile_pool(name="fold", bufs=2))
        red_pool = ctx.enter_context(tc.tile_pool(name="red", bufs=2))
        pp_pool = ctx.enter_context(tc.tile_pool(name="pp", bufs=2))
        x_pool = ctx.enter_context(tc.tile_pool(name="x", bufs=2))
        h2_pool = ctx.enter_context(tc.tile_pool(name="h2", bufs=2))
        ff_pool = ctx.enter_context(tc.tile_pool(name="ff", bufs=2))
        ep_pool = ctx.enter_context(tc.tile_pool(name="ep", bufs=2))
        ps_h2 = ctx.enter_context(tc.tile_pool(name="ps_h2", bufs=2, space="PSUM"))
        ps_w3 = ctx.enter_context(tc.tile_pool(name="ps_w3", bufs=1, space="PSUM"))
        ps_pp = ctx.enter_context(tc.tile_pool(name="ps_pp", bufs=1, space="PSUM"))
        ps_ft = ctx.enter_context(tc.tile_pool(name="ps_ft", bufs=1, space="PSUM"))

        def load_quarters(c):
            # One pair (2 batches) as 4 x 2MB h-quarter tiles; even
            # quarters ride SP, odd ACT -> both rings stay fed.
            qs = []
            for q in range(4):
                tft = tf_pool.tile([128, HQ, T], f32, name=f"tf{c}_{q}",
                                   tag="tft")
                eng = nc.sync if q % 2 == 0 else nc.scalar
                eng.dma_start(
                    tft[:], tf[2 * c : 2 * c + 2, :, q * HQ : (q + 1) * HQ, :])
                qs.append(tft[:])
            return qs

        # tf streaming starts immediately; consts follow on the ACT ring
        # (small, land within ~1us, needed only ~15us in).
        pending = load_quarters(0)
        cb_sb = consts.tile([128, 384], bf16)
        nc.scalar.dma_start(cb_sb[:], CB[:])
        cf_sb = consts.tile([128, 132], f32)
        nc.scalar.dma_start(cf_sb[:], CF[:])

        wi_sb = cb_sb[:, CB_WI : CB_WI + 128]
        wj_sb = cb_sb[:, CB_WJ : CB_WJ + 128]
        w2_sb = cb_sb[:, CB_W2 : CB_W2 + 64]
        w3_sb = cb_sb[0:64, CB_W3 : CB_W3 + 64]
        b1_sb = cf_sb[:, CF_B1 : CF_B1 + 1]
        b2_sb = cf_sb[0:64, CF_B2 : CF_B2 + 1]
        b3_sb = cf_sb[0:8, CF_B3 : CF_B3 + 1]
        g_sb = cf_sb[0:64, CF_G : CF_G + 64]
        i64_sb = cf_sb[0:64, CF_I64 : CF_I64 + 64]
        zro_sb = cf_sb[0:64, CF_Z : CF_Z + 1]

        def emit_fold_chain(mode, th, r_sb, h0, c, tag):
            # Sum over t for one h-quarter: R[p=(b_lo,n), h0:h0+HQ].
            # First fold reads f32, outputs bf16 (1x); the bf16 tail runs
            # in DVE 2x/4x mode. mode 'g1' puts the f32 fold1 on Pool.
            eng1 = nc.gpsimd if mode == 'g1' else nc.vector
            f1 = fold_pool.tile([128, HQ, 64], bf16, tag=f"f1{tag}",
                                name=f"f1_{c}_{h0}")
            eng1.tensor_tensor(f1[:], th[:, :, 0:64], th[:, :, 64:128],
                               op=ALU.add)
            f2 = fold_pool.tile([128, HQ, 32], bf16, tag=f"f2{tag}",
                                name=f"f2_{c}_{h0}")
            nc.vector.tensor_tensor(f2[:], f1[:, :, 0:32], f1[:, :, 32:64],
                                    op=ALU.add)
            f3 = fold_pool.tile([128, HQ, 16], bf16, tag=f"f3{tag}",
                                name=f"f3_{c}_{h0}")
            nc.vector.tensor_tensor(f3[:], f2[:, :, 0:16], f2[:, :, 16:32],
                                    op=ALU.add)
            nc.vector.reduce_sum(r_sb[:, h0 : h0 + HQ], f3[:], axis=AX.X)

        for c in range(PAIRS):
            parts = pending
            if c + 1 < PAIRS:
                pending = load_quarters(c + 1)

            r_sb = red_pool.tile([128, H], bf16, tag="r", name=f"r{c}")
            for q in range(4):
                emit_fold_chain(ENG_QUARTER[q], parts[q], r_sb, q * HQ, c,
                                ENG_QUARTER[q] + str(q))

            # node_T[h, (b_lo, n)] via DMA XBAR transpose (bf16)
            rt_sb = red_pool.tile([H, 128], bf16, tag="rt", name=f"rt{c}")
            nc.sync.dma_start_transpose(rt_sb[:], r_sb[:])

            # pi = Wi.T @ node_T (+b1 on evac), pj = Wj.T @ node_T
            pp_ps = ps_pp.tile([128, 256], f32, tag="pp")
            nc.tensor.matmul(pp_ps[:, 0:128], wi_sb, rt_sb[:],
                             start=True, stop=True)
            nc.tensor.matmul(pp_ps[:, 128:256], wj_sb, rt_sb[:],
                             start=True, stop=True)
            pi_sb = pp_pool.tile([H, 2, 64], bf16, tag="pi", name=f"pi{c}")
            if NOBIAS:
                nc.scalar.activation(pi_sb[:], pp_ps[:, 0:128], AF.Copy)
            else:
                nc.scalar.activation(pi_sb[:], pp_ps[:, 0:128], AF.Identity,
                                     bias=b1_sb)
            pj_sb = pp_pool.tile([H, 2, 64], bf16, tag="pj", name=f"pj{c}")
            nc.scalar.activation(pj_sb[:], pp_ps[:, 128:256], AF.Copy)

            w3_ps = [ps_w3.tile([8, 512], f32, tag=f"w3_{b}",
                                name=f"w3_{c}_{b}") for b in range(2)]
            for c2 in range(NCH):
                # X_pre = pi[:, b, i] + pj[:, b, j] for chunk's 8 i values,
                # both batches: (128, 2, 8, 64) in one op
                xa = x_pool.tile([H, 2, 8, 64], bf16, tag="xa",
                                 name=f"xa_{c}_{c2}")
                xeng = nc.vector if XADD_ENG[c2] == 'v' else nc.gpsimd
                xeng.tensor_tensor(
                    xa[:],
                    pi_sb[:, :, 8 * c2 : 8 * c2 + 8].unsqueeze(3)
                        .broadcast_to((H, 2, 8, 64)),
                    pj_sb[:].unsqueeze(2).broadcast_to((H, 2, 8, 64)),
                    op=ALU.add)
                x_sb = x_pool.tile([H, 2, 8, 64], bf16, tag="x",
                                   name=f"x_{c}_{c2}")
                nc.scalar.activation(x_sb[:], xa[:], AF.Relu)
                h2_ps = ps_h2.tile([64, 1024], f32, tag="h2ps")
                nc.tensor.matmul(h2_ps[:, 0:512], w2_sb, x_sb[:, 0],
                                 start=True, stop=True)
                nc.tensor.matmul(h2_ps[:, 512:1024], w2_sb, x_sb[:, 1],
                                 start=True, stop=True)
                h2_sb = h2_pool.tile([64, 1024], bf16, tag="h2",
                                     name=f"h2_{c}_{c2}")
                if H2_ENG[c2] == 'v':
                    # relu(x + b2) == max(x, -b2) + b2; with zero bias a
                    # single max suffices
                    if NOBIAS:
                        nc.vector.tensor_scalar(
                            h2_sb[:], h2_ps[:], scalar1=0.0, scalar2=None,
                            op0=ALU.max)
                    else:
                        nc.vector.tensor_scalar(
                            h2_sb[:], h2_ps[:], scalar1=b2_sb, scalar2=0.0,
                            op0=ALU.add, op1=ALU.max)
                else:
                    nc.scalar.activation(h2_sb[:], h2_ps[:], AF.Relu,
                                         bias=0.0 if NOBIAS else b2_sb)
                for b in range(2):
                    nc.tensor.matmul(
                        w3_ps[b][:],
                        w3_sb[:, 8 * c2 : 8 * c2 + 8],
                        h2_sb[:, 512 * b : 512 * b + 512],
                        start=(c2 == 0), stop=(c2 == NCH - 1))

            # Epilogue, both batches fused as (64, 2, 64) tiles:
            # out = (relu(G + F + F^T) + I) / (rowsum + 1e-8)
            f_sb = ep_pool.tile([N, 2, N], f32, tag="f")
            ft_ps = ps_ft.tile([N, 2, N], f32, tag="ft")
            for b_lo in range(2):
                # F_flat -> F (64, 64): same linearized element order
                ff_sb = ff_pool.tile([8, 512], f32, tag="ff",
                                     name=f"ff_{c}_{b_lo}")
                if FF_ENG == 'v':
                    nc.vector.tensor_scalar(
                        ff_sb[:], w3_ps[b_lo][:],
                        scalar1=0.0 if NOBIAS else b3_sb, scalar2=None,
                        op0=ALU.add)
                elif NOBIAS:
                    nc.scalar.activation(ff_sb[:], w3_ps[b_lo][:], AF.Copy)
                else:
                    nc.scalar.activation(ff_sb[:], w3_ps[b_lo][:], AF.Identity,
                                         bias=b3_sb)
                nc.sync.dma_start(f_sb[:, b_lo], ff_sb[:])
                nc.tensor.transpose(ft_ps[:, b_lo], f_sb[:, b_lo], i64_sb)
            t1 = ep_pool.tile([N, 2, N], f32, tag="t1")
            nc.vector.tensor_tensor(t1[:], f_sb[:], ft_ps[:], op=ALU.add)
            t2 = ep_pool.tile([N, 2, N], f32, tag="t2")
            nc.vector.tensor_tensor(
                t2[:], t1[:],
                g_sb.unsqueeze(1).broadcast_to((N, 2, N)), op=ALU.add)
            sp = ep_pool.tile([N, 2, N], f32, tag="sp")
            nc.vector.tensor_scalar(
                sp[:], t2[:], scalar1=0.0, scalar2=None, op0=ALU.max)
            spi = ep_pool.tile([N, 2, N], f32, tag="spi")
            nc.vector.tensor_tensor(
                spi[:], sp[:],
                i64_sb.unsqueeze(1).broadcast_to((N, 2, N)), op=ALU.add)
            rs = ep_pool.tile([N, 2], f32, tag="rs")
            nc.vector.reduce_sum(rs[:], spi[:], axis=AX.X)
            rb = ep_pool.tile([N, 2], f32, tag="rb")
            nc.vector.tensor_scalar(
                rb[:], rs[:], scalar1=1e-8, scalar2=None, op0=ALU.add)
            rec = ep_pool.tile([N, 2], f32, tag="rec")
            nc.vector.reciprocal(rec[:], rb[:])
            for b_lo in range(2):
                o_sb = ep_pool.tile([N, N], f32, tag=f"o{b_lo}",
                                    name=f"o_{c}_{b_lo}")
                nc.vector.tensor_scalar(
                    o_sb[:], spi[:, b_lo], scalar1=rec[:, b_lo : b_lo + 1],
                    scalar2=None, op0=ALU.mult)
                nc.sync.dma_start(out_ext[2 * c + b_lo], o_sb[:])
    return nc


def _get_nc():
    key = ('nc', _CACHE.get('cfg_nobias', False))
    if key not in _CACHE:
        _CACHE[key] = build_nc()
    return _CACHE[key]


def kernel(**inputs):
    import ml_dtypes

    from concourse.bass_utils import run_bass_kernel_spmd

    _install_wait_splitter()

    tf = np.asarray(inputs["temporal_features"], dtype=np.float32)
    ap = np.asarray(inputs["adj_param"], dtype=np.float32)
    W1 = np.asarray(inputs["W1"], dtype=np.float32)
    b1 = np.asarray(inputs["b1"], dtype=np.float32)
    W2 = np.asarray(inputs["W2"], dtype=np.float32)
    b2 = np.asarray(inputs["b2"], dtype=np.float32)
    W3 = np.asarray(inputs["W3"], dtype=np.float32)
    b3 = np.asarray(inputs["b3"], dtype=np.float32)

    bf = ml_dtypes.bfloat16
    # Per chunk, an (H//2, 8) one-hot-column weight routing the chunk's
    # scalar output to PSUM partition `chunk` (0.25 sym factor folded in).
    W3blk = np.zeros((H // 2, NCH, 8), np.float32)
    for chunk in range(NCH):
        W3blk[:, chunk, chunk] = 0.25 * W3[:, 0]

    CBnp = np.zeros((128, 384), np.float32)
    CBnp[:, CB_WI : CB_WI + 128] = W1[:H] / T
    CBnp[:, CB_WJ : CB_WJ + 128] = W1[H:] / T
    CBnp[:, CB_W2 : CB_W2 + 64] = W2
    CBnp[0:64, CB_W3 : CB_W3 + 64] = W3blk.reshape(H // 2, 8 * NCH)
    CBnp = np.ascontiguousarray(CBnp.astype(bf))

    CFnp = np.zeros((128, 132), np.float32)
    CFnp[:, CF_B1] = b1
    CFnp[0:64, CF_B2] = b2
    CFnp[0:8, CF_B3] = 0.25 * float(b3[0])
    CFnp[0:64, CF_G : CF_G + 64] = 0.25 * (ap + ap.T)
    CFnp[0:64, CF_I64 : CF_I64 + 64] = np.eye(N, dtype=np.float32)
    CFnp = np.ascontiguousarray(CFnp)

    shared = {"CB": CBnp, "CF": CFnp}
    in_maps = [
        {"tf": np.ascontiguousarray(tf[i * B_LOC : (i + 1) * B_LOC]), **shared}
        for i in range(NCORES)
    ]

    _CACHE['cfg_nobias'] = bool(
        not b1.any() and not b2.any() and not b3.any())
    nc = _get_nc()
    res = run_bass_kernel_spmd(nc, in_maps, core_ids=list(range(NCORES)),
                               **_CACHE.get('run_kwargs', {}))
    _CACHE['last_result'] = res
    out = np.concatenate([res.results[i]["out"] for i in range(NCORES)], axis=0)
    return np.ascontiguousarray(out.astype(np.float32))
